# revision 1
# baseline (speedup 1.0000x reference)
"""MoE-routed per-sample conv2d kernel for Trainium2 (8 NeuronCores, SPMD).

Math (per sample b):
    y_ctx  = mean(y[b], HW)                              [C]
    gates  = softmax(y_ctx @ (gate_w[:C] + gate_w[C:]) + gate_b)   [E]
    Wf[e]  = experts[e,:, :C] + experts[e,:, C:]         [O, C, K, K]  (fold of q;q concat)
    agg    = sum_e gates[e] * Wf[e]
    out[b] = conv2d(q[b], agg, SAME)

Sharding: data-parallel over batch. Each of the 8 cores handles B/8 = 2
samples; experts/gate params replicated. Conv runs on the TensorEngine as
9 shifted matmuls (one per kernel tap) accumulated in PSUM, fp32r.
"""

import numpy as np

import concourse.bass as bass
import concourse.tile as tile
from concourse import bacc, mybir
from concourse.bass_utils import run_bass_kernel_spmd
from concourse.masks import make_identity
from concourse.tile_rust import add_dep_helper

F32 = mybir.dt.float32
F32R = mybir.dt.float32r

B, C, O, H, W, E, K = 16, 128, 128, 128, 128, 3, 3
NCORES = 8
BPC = B // NCORES          # samples per core
CH_ROWS = 16               # output rows per conv chunk
NCH = H // CH_ROWS         # chunks per sample
RB_ROWS = 4                # output rows per PSUM block (4*128 = 512 free)
NRB = CH_ROWS // RB_ROWS   # row blocks per chunk
XCF = 2 + (CH_ROWS + 3) * W      # flat chunk tile: 2 lead zeros, 34 rows, slack row
YCHUNK = 2048              # y columns per reduce chunk
NYCH = (H * W) // YCHUNK

# taps ordered so the first one covers the full output range (ky=1,kx=1)
TAPS = [(1, 1)] + [(ky, kx) for ky in range(3) for kx in range(3) if (ky, kx) != (1, 1)]


def build_nc():
    nc = bacc.Bacc(None, target_bir_lowering=False)

    q_d = nc.dram_tensor("q", [BPC, C, H, W], F32, kind="ExternalInput")
    y_d = nc.dram_tensor("y", [BPC, C, H, W], F32, kind="ExternalInput")
    ex_d = nc.dram_tensor("experts", [E, O, 2 * C, K, K], F32, kind="ExternalInput")
    gw_d = nc.dram_tensor("gate_w", [2 * C, E], F32, kind="ExternalInput")
    gb_d = nc.dram_tensor("gate_b", [E], F32, kind="ExternalInput")
    out_d = nc.dram_tensor("out", [BPC, O, H, W], F32, kind="ExternalOutput")

    with tile.TileContext(nc) as tc:
        import contextlib

        with contextlib.ExitStack() as ctx:
            const = ctx.enter_context(tc.tile_pool(name="const", bufs=1))
            wraw = ctx.enter_context(tc.tile_pool(name="wraw", bufs=2))
            wft = ctx.enter_context(tc.tile_pool(name="wft", bufs=3))
            ypool = ctx.enter_context(tc.tile_pool(name="ypool", bufs=8))
            gp = ctx.enter_context(tc.tile_pool(name="gp", bufs=4))
            atmp = ctx.enter_context(tc.tile_pool(name="atmp", bufs=1))
            aggp = ctx.enter_context(tc.tile_pool(name="aggp", bufs=2))
            xcp = ctx.enter_context(tc.tile_pool(name="xcp", bufs=6))
            osbp = ctx.enter_context(tc.tile_pool(name="osbp", bufs=4))
            psp = ctx.enter_context(tc.tile_pool(name="psp", bufs=8, space="PSUM"))

            # two HWDGE rings (SP + ACT); stripe bulk DMAs across both
            ring_state = [0]

            def ring():
                ring_state[0] += 1
                return nc.sync if ring_state[0] % 2 == 0 else nc.scalar

            # per-ring FIFO chaining for the startup section: without this the
            # scheduler happily floats "ready" q-chunk loads ahead of y chunks
            # whose DMA still waits on a pool slot, starving the gating path
            last_dma = {}
            chain_on = [True]

            def chained_dma(eng, out, in_):
                inst = eng.dma_start(out=out, in_=in_)
                if chain_on[0]:
                    key = eng.engine
                    if key in last_dma:
                        add_dep_helper(inst.ins, last_dma[key], sync=False,
                                       reason="ring FIFO order")
                    last_dma[key] = inst.ins
                return inst

            # ---- constants -------------------------------------------------
            ident = const.tile([128, 128], F32, tag="ident", name="ident")
            make_identity(nc, ident)

            # prewarm the ACT Exp table so gating doesn't pay the table load
            warm = const.tile([1, 1], F32, tag="warm", name="warm")
            nc.vector.memset(warm[:], 0.0)
            nc.scalar.activation(warm[:], warm[:], mybir.ActivationFunctionType.Exp,
                                 bias=0.0, scale=1.0)

            ones = const.tile([1, 128], F32, tag="ones", name="ones")
            nc.vector.memset(ones[:], 1.0)

            gw = const.tile([C, 2, E], F32, tag="gw", name="gw")
            nc.gpsimd.dma_start(gw[:], gw_d[:].rearrange("(h c) e -> c h e", h=2))
            weff = const.tile([C, E], F32, tag="weff", name="weff")
            nc.vector.tensor_add(weff[:], gw[:, 0, :], gw[:, 1, :])
            # fold the 1/HW of the y-mean into the gate weight
            nc.vector.tensor_scalar_mul(weff[:], weff[:], 1.0 / float(H * W))

            gbt = const.tile([1, E], F32, tag="gbt", name="gbt")
            nc.gpsimd.dma_start(gbt[:], gb_d[:].rearrange("(x e) -> x e", x=1))

            # ---- expert fold + transpose to [c, t, o] ----------------------
            # (emitted first so the small expert DMAs beat the bulk y/q
            # traffic onto the ring and the PE transposes can start early)
            # raw layout per expert: [o, i, ky, kx], i in [0, 2C)
            wfts = []
            for e, eng in ((0, nc.sync), (1, nc.scalar), (2, nc.sync)):
                we = wraw.tile([O, 2 * C, K, K], F32, tag="wraw", name=f"we{e}")
                chained_dma(eng, we[:], ex_d[e])
                # fold the two input-channel halves in place
                nc.vector.tensor_add(we[:, 0:C, :, :], we[:, 0:C, :, :],
                                     we[:, C:2 * C, :, :])
                wt = wft.tile([C, K * K, O], F32, tag="wft", name=f"wft{e}")
                for t, (ky, kx) in enumerate(TAPS):
                    pst = psp.tile([128, 128], F32, tag="ps", name=f"pst{e}_{t}")
                    nc.tensor.transpose(pst[:], we[:, 0:C, ky, kx], ident[:])
                    # keep these off DVE: DVE is in-order and the y reduces
                    # queued behind PE-dependent copies would stall the gates
                    nc.scalar.copy(wt[:, t, :], pst[:])
                wfts.append(wt)

            # ---- y reduction -----------------------------------------------
            yflat = y_d[:].rearrange("b c h w -> b c (h w)")
            ysums = []

            def reduce_y(b, nsync=None):
                # stripe the chunks across both HWDGE rings (nsync of them on
                # the sync ring); reduce on DVE (sync half) and on ACT via
                # activation-accumulate (scalar half) in parallel
                if nsync is None:
                    nsync = NYCH // 2
                ypart = gp.tile([C, NYCH], F32, tag="ypart", name=f"ypart{b}")
                order = []
                a, bb = 0, nsync
                while a < nsync or bb < NYCH:
                    if a < nsync:
                        order.append((a, nc.sync)); a += 1
                    if bb < NYCH:
                        order.append((bb, nc.scalar)); bb += 1
                for j, eng in order:
                    yc = ypool.tile([C, YCHUNK], F32, tag="yc", name=f"yc{b}_{j}")
                    chained_dma(eng, yc[:], yflat[b, :, j * YCHUNK:(j + 1) * YCHUNK])
                    if j < nsync:
                        nc.vector.reduce_sum(ypart[:, j:j + 1], yc[:],
                                             axis=mybir.AxisListType.X)
                    else:
                        nc.scalar.activation(
                            yc[:], yc[:], mybir.ActivationFunctionType.Copy,
                            accum_out=ypart[:, j:j + 1])
                ysum = gp.tile([C, 1], F32, tag="ysum", name=f"ysum{b}")
                nc.vector.reduce_sum(ysum[:], ypart[:], axis=mybir.AxisListType.X)
                ysums.append(ysum)

            # ---- q chunk staging -------------------------------------------
            # chunk tile: [C, 34, 130]; row j holds x row (32*ch - 1 + j),
            # col s holds x col (s - 1); zero borders for SAME padding.
            xcs = {}

            def load_xc(b, ch, eng=None):
                # Fully contiguous load: flat [2 zeros][row -1..row 32][2 zeros],
                # rows packed at stride W (no column padding). Column wrap-around
                # is fixed up by the edge-correction matmuls in conv_chunk.
                xr_lo = max(0, CH_ROWS * ch - 1)
                xr_hi = min(H - 1, CH_ROWS * ch + CH_ROWS)
                nrows = xr_hi - xr_lo + 1
                j0 = xr_lo - (CH_ROWS * ch - 1)
                xc = xcp.tile([C, XCF], F32R, tag="xc", name=f"xc{b}_{ch}")
                nc.gpsimd.memset(xc[:, 0:2].bitcast(F32), 0.0)
                nc.gpsimd.memset(
                    xc[:, 2 + (CH_ROWS + 2) * W: 2 + (CH_ROWS + 2) * W + 2].bitcast(F32), 0.0)
                if ch == 0:
                    nc.gpsimd.memset(xc[:, 2:2 + W].bitcast(F32), 0.0)
                if ch == NCH - 1:
                    nc.gpsimd.memset(
                        xc[:, 2 + (CH_ROWS + 1) * W: 2 + (CH_ROWS + 2) * W].bitcast(F32), 0.0)
                chained_dma(
                    eng or ring(),
                    xc[:, 2 + j0 * W: 2 + (j0 + nrows) * W],
                    q_d[b, :, xr_lo:xr_hi + 1, :].rearrange("c h w -> c (h w)").bitcast(F32R),
                )
                xcs[(b, ch)] = xc

            # ---- gating + weight aggregation per sample --------------------
            aggs = []

            def gate_and_agg(b):
                ps13 = psp.tile([1, E], F32, tag="ps", name=f"ps13_{b}")
                nc.tensor.matmul(ps13[:], ysums[b][:], weff[:], start=True, stop=True)
                logits = gp.tile([1, E], F32, tag="logits", name=f"logits{b}")
                nc.vector.tensor_add(logits[:], ps13[:], gbt[:])
                mx = gp.tile([1, 1], F32, tag="mx", name=f"mx{b}")
                nc.vector.reduce_max(mx[:], logits[:], axis=mybir.AxisListType.X)
                nc.vector.tensor_scalar_mul(mx[:], mx[:], -1.0)
                nc.scalar.activation(logits[:], logits[:], mybir.ActivationFunctionType.Exp,
                                     bias=mx[:], scale=1.0)
                sm = gp.tile([1, 1], F32, tag="sm", name=f"sm{b}")
                nc.vector.reduce_sum(sm[:], logits[:], axis=mybir.AxisListType.X)
                nc.vector.reciprocal(sm[:], sm[:])
                nc.vector.tensor_scalar_mul(logits[:], logits[:], sm[:])
                # broadcast gates to all partitions via a K=1 matmul with ones
                psg = psp.tile([128, E], F32, tag="ps", name=f"psg{b}")
                nc.tensor.matmul(psg[:], ones[:], logits[:], start=True, stop=True)
                gbc = gp.tile([128, E], F32, tag="gbc", name=f"gbc{b}")
                nc.vector.tensor_copy(gbc[:], psg[:])

                # aggregate in 3 tap-groups so the first conv matmuls (tap 0)
                # can start while the rest of the weights are still combining
                accf = atmp.tile([C, K * K, O], F32, tag="accf", name=f"accf{b}")
                tmp = atmp.tile([C, K * K, O], F32, tag="tmp", name=f"tmp{b}")
                agg = aggp.tile([C, K * K, O], F32R, tag="agg", name=f"agg{b}")
                for g3 in range(3):
                    sl = slice(3 * g3, 3 * g3 + 3)
                    nc.vector.tensor_scalar_mul(accf[:, sl, :], wfts[0][:, sl, :], gbc[:, 0:1])
                    nc.vector.tensor_scalar_mul(tmp[:, sl, :], wfts[1][:, sl, :], gbc[:, 1:2])
                    nc.vector.tensor_add(accf[:, sl, :], accf[:, sl, :], tmp[:, sl, :])
                    nc.vector.tensor_scalar_mul(tmp[:, sl, :], wfts[2][:, sl, :], gbc[:, 2:3])
                    nc.vector.tensor_add(accf[:, sl, :], accf[:, sl, :], tmp[:, sl, :])
                    nc.vector.tensor_copy(agg[:, sl, :], accf[:, sl, :])
                aggs.append(agg)

            # ---- conv ------------------------------------------------------
            # Main taps read the flat chunk at offset 2 + (4rb+ky)*W + kx-1.
            # For kx=0 the first column of each row wrongly reads the last
            # element of the previous row (and vice versa for kx=2), which
            # SAME-padding says should be zero.  err matmuls compute exactly
            # those wrong contributions; they are subtracted during PSUM->SBUF.
            def conv_chunk(b, ch):
                xc = xcs[(b, ch)]
                # shifted row views: x1[c, r, w] = flat[1 + r*W + w],
                #                    x2[c, r, w] = flat[2 + r*W + w]
                x1 = xc[:, 1:1 + (CH_ROWS + 2) * W].rearrange("c (r w) -> c r w", w=W)
                x2 = xc[:, 2:2 + (CH_ROWS + 3) * W].rearrange("c (r w) -> c r w", w=W)
                # err psum [O, 2, CH_ROWS]: group 0 = col 0, group 1 = col W-1
                errps = psp.tile([O, 2, CH_ROWS], F32, tag="ps", name=f"eps{b}_{ch}")
                first = True
                for t, (ky, kx) in enumerate(TAPS):
                    if kx == 1:
                        continue
                    if kx == 0:
                        # out col 0 wrongly reads flat[(row+ky)*W + 1]
                        g, rhs = 0, x1[:, ky:ky + CH_ROWS, 0:1]
                    else:
                        # out col W-1 wrongly reads flat[2 + (row+ky+1)*W]
                        g, rhs = 1, x2[:, ky + 1:ky + 1 + CH_ROWS, 0:1]
                    nc.tensor.matmul(
                        errps[:, g, :], aggs[b][:, t, :], rhs,
                        start=first, stop=(t == len(TAPS) - 1), skip_group_check=True,
                    )
                    first = False
                for rb in range(NRB):
                    r0 = CH_ROWS * ch + RB_ROWS * rb
                    ps = psp.tile([O, RB_ROWS, W], F32, tag="ps", name=f"ps{b}_{ch}_{rb}")
                    for t, (ky, kx) in enumerate(TAPS):
                        jb = RB_ROWS * rb + ky  # tile row of x row r0+ky-1
                        off = 2 + jb * W + kx - 1
                        rhs = xc[:, off:off + RB_ROWS * W]  # contiguous 512
                        nc.tensor.matmul(
                            ps[:],
                            aggs[b][:, t, :],
                            rhs,
                            start=(t == 0),
                            stop=(t == len(TAPS) - 1),
                        )
                    osb = osbp.tile([O, RB_ROWS, W], F32, tag="osb", name=f"osb{b}_{ch}_{rb}")
                    if rb % 2 == 0:
                        nc.vector.tensor_copy(osb[:], ps[:])
                    else:
                        nc.scalar.copy(osb[:], ps[:])
                    sl = slice(RB_ROWS * rb, RB_ROWS * (rb + 1))
                    nc.vector.tensor_sub(osb[:, :, 0], osb[:, :, 0], errps[:, 0, sl])
                    nc.vector.tensor_sub(osb[:, :, W - 1], osb[:, :, W - 1], errps[:, 1, sl])
                    ring().dma_start(out_d[b, :, r0:r0 + RB_ROWS, :], osb[:])

            # ---- schedule --------------------------------------------------
            # Emission order doubles as per-engine program order; keep the
            # sample-0 gating chain (y0 -> gates0 -> agg0) unobstructed on
            # DVE and get the first q chunks onto the rings right behind y0.
            reduce_y(0, nsync=3)   # sync ring also carries 2 experts
            gate_and_agg(0)
            load_xc(0, 0, nc.sync)
            load_xc(0, 1, nc.scalar)
            load_xc(0, 2, nc.sync)
            load_xc(0, 3, nc.scalar)
            chain_on[0] = False   # steady state: let the scheduler pack freely
            conv_chunk(0, 0)
            load_xc(0, 4, nc.sync)
            load_xc(0, 5, nc.scalar)
            conv_chunk(0, 1)
            load_xc(0, 6, nc.sync)
            load_xc(0, 7, nc.scalar)
            conv_chunk(0, 2)
            conv_chunk(0, 3)
            reduce_y(1)          # y1 rides the rings behind sample-0's chunks
            conv_chunk(0, 4)
            load_xc(1, 0, nc.sync)
            load_xc(1, 1, nc.scalar)
            conv_chunk(0, 5)
            gate_and_agg(1)
            pending = [(1, ch) for ch in range(2, NCH)]
            todo = [(0, ch) for ch in range(6, NCH)] + \
                   [(1, ch) for ch in range(NCH)]
            li = 0
            for k, (b, ch) in enumerate(todo):
                if li < len(pending):
                    load_xc(*pending[li], nc.sync)
                    li += 1
                if li < len(pending):
                    load_xc(*pending[li], nc.scalar)
                    li += 1
                conv_chunk(b, ch)

    nc.compile()
    return nc


_NC_CACHE = None


def kernel(q, y, experts, gate_w, gate_b, _trace=False, _result_box=None):
    global _NC_CACHE
    if _NC_CACHE is None:
        _NC_CACHE = build_nc()
    nc = _NC_CACHE

    q = np.ascontiguousarray(q, dtype=np.float32)
    y = np.ascontiguousarray(y, dtype=np.float32)
    experts = np.ascontiguousarray(experts, dtype=np.float32)
    gate_w = np.ascontiguousarray(gate_w, dtype=np.float32)
    gate_b = np.ascontiguousarray(gate_b, dtype=np.float32)

    in_maps = []
    for i in range(NCORES):
        sl = slice(i * BPC, (i + 1) * BPC)
        in_maps.append({
            "q": q[sl], "y": y[sl],
            "experts": experts, "gate_w": gate_w, "gate_b": gate_b,
        })

    kwargs = {}
    if _trace:
        kwargs = dict(trace=True, trace_cores=[0])
    res = run_bass_kernel_spmd(nc, in_maps, core_ids=list(range(NCORES)), **kwargs)
    if _result_box is not None:
        _result_box.append(res)
    return np.concatenate([res.results[i]["out"] for i in range(NCORES)], axis=0)



# revision 4
# speedup vs baseline: 1.1253x; 1.1253x over previous
"""MoE-routed per-sample conv2d kernel for Trainium2 (8 NeuronCores, SPMD).

Math (per sample b):
    y_ctx  = mean(y[b], HW)                              [C]
    gates  = softmax(y_ctx @ (gate_w[:C] + gate_w[C:]) + gate_b)   [E]
    Wf[e]  = experts[e,:, :C] + experts[e,:, C:]         [O, C, K, K]  (fold of q;q concat)
    agg    = sum_e gates[e] * Wf[e]
    out[b] = conv2d(q[b], agg, SAME)

Sharding: data-parallel over batch. Each of the 8 cores handles B/8 = 2
samples; experts/gate params replicated. Conv runs on the TensorEngine as
9 shifted matmuls (one per kernel tap) accumulated in PSUM, fp32r.

Engine roles:
  SP (sync)      bulk-load DMA queue A (experts/y/q), FIFO-chained
  Pool (gpsimd)  bulk-load DMA queue B (SWDGE) + xc guard memsets + tiny loads
  DVE (vector)   y reduces, gating vector ops, agg combine (FMA), err subs
  ACT (scalar)   exp, wft drains, all PSUM->SBUF output copies, and the
                 output-write DMA triggers (its HWDGE queue carries no loads,
                 so writes never sit behind a blocked prefetch)
  PE (tensor)    folded expert transposes (accumulating), gating matmuls, conv
"""

import numpy as np

import concourse.bass as bass
import concourse.tile as tile
from concourse import bacc, mybir
from concourse.bass_utils import run_bass_kernel_spmd
from concourse.masks import make_identity
from concourse.tile_rust import add_dep_helper

F32 = mybir.dt.float32
F32R = mybir.dt.float32r

B, C, O, H, W, E, K = 16, 128, 128, 128, 128, 3, 3
NCORES = 8
BPC = B // NCORES          # samples per core
CH_ROWS = 16               # output rows per conv chunk
NCH = H // CH_ROWS         # chunks per sample
RB_ROWS = 4                # output rows per PSUM block (4*128 = 512 free)
NRB = CH_ROWS // RB_ROWS   # row blocks per chunk
XCF = 2 + (CH_ROWS + 3) * W      # flat chunk tile: 2 lead zeros, 19 rows, slack
YCHUNK = 1024              # y columns per reduce chunk (0.5 MB)
NYCH = (H * W) // YCHUNK   # 16

# taps ordered so the first one covers the full output range (ky=1,kx=1)
TAPS = [(1, 1)] + [(ky, kx) for ky in range(3) for kx in range(3) if (ky, kx) != (1, 1)]

MUL = mybir.AluOpType.mult
ADD = mybir.AluOpType.add


def build_nc():
    nc = bacc.Bacc(None, target_bir_lowering=False)

    q_d = nc.dram_tensor("q", [BPC, C, H, W], F32, kind="ExternalInput")
    y_d = nc.dram_tensor("y", [BPC, C, H, W], F32, kind="ExternalInput")
    ex_d = nc.dram_tensor("experts", [E, O, 2 * C, K, K], F32, kind="ExternalInput")
    gw_d = nc.dram_tensor("gate_w", [2 * C, E], F32, kind="ExternalInput")
    gb_d = nc.dram_tensor("gate_b", [E], F32, kind="ExternalInput")
    out_d = nc.dram_tensor("out", [BPC, O, H, W], F32, kind="ExternalOutput")

    with tile.TileContext(nc) as tc:
        import contextlib

        with contextlib.ExitStack() as ctx:
            const = ctx.enter_context(tc.tile_pool(name="const", bufs=1))
            wraw = ctx.enter_context(tc.tile_pool(name="wraw", bufs=3))
            wft = ctx.enter_context(tc.tile_pool(name="wft", bufs=3))
            ypool = ctx.enter_context(tc.tile_pool(name="ypool", bufs=8))
            gp = ctx.enter_context(tc.tile_pool(name="gp", bufs=4))
            atmp = ctx.enter_context(tc.tile_pool(name="atmp", bufs=1))
            aggp = ctx.enter_context(tc.tile_pool(name="aggp", bufs=2))
            xcp = ctx.enter_context(tc.tile_pool(name="xcp", bufs=6))
            osbp = ctx.enter_context(tc.tile_pool(name="osbp", bufs=3))
            psp = ctx.enter_context(tc.tile_pool(name="psp", bufs=6, space="PSUM"))
            pse = ctx.enter_context(tc.tile_pool(name="pse", bufs=2, space="PSUM"))

            # Two bulk-load rings: SP (HWDGE) and Pool/gpsimd (SWDGE). Keep
            # each ring's transfer order exactly as emitted: the static Tile
            # scheduler otherwise floats "ready" q-chunk loads ahead of y
            # chunks whose DMA waits on a pool slot, starving the gating path.
            last_dma = {}

            def chained_dma(eng, out, in_):
                inst = eng.dma_start(out=out, in_=in_)
                key = eng.engine
                if key in last_dma:
                    add_dep_helper(inst.ins, last_dma[key], sync=False,
                                   reason="ring FIFO order")
                last_dma[key] = inst.ins
                return inst

            # ---- tiny loads + constants ------------------------------------
            gw = const.tile([C, 2, E], F32, tag="gw", name="gw")
            chained_dma(nc.gpsimd, gw[:], gw_d[:].rearrange("(h c) e -> c h e", h=2))
            gbt = const.tile([1, E], F32, tag="gbt", name="gbt")
            chained_dma(nc.gpsimd, gbt[:], gb_d[:].rearrange("(x e) -> x e", x=1))

            # expert loads: e0 on SP; e1, e2 on gpsimd ring
            wes = []
            for e, eng in ((0, nc.sync), (1, nc.gpsimd), (2, nc.gpsimd)):
                we = wraw.tile([O, 2 * C, K, K], F32, tag="wraw", name=f"we{e}")
                chained_dma(eng, we[:], ex_d[e])
                wes.append(we)

            ident = const.tile([128, 128], F32, tag="ident", name="ident")
            make_identity(nc, ident)

            # prewarm the ACT Exp table so gating doesn't pay the table load
            warm = const.tile([1, 1], F32, tag="warm", name="warm")
            nc.vector.memset(warm[:], 0.0)
            nc.scalar.activation(warm[:], warm[:], mybir.ActivationFunctionType.Exp,
                                 bias=0.0, scale=1.0)

            ones = const.tile([1, 128], F32, tag="ones", name="ones")
            nc.vector.memset(ones[:], 1.0)

            weff = const.tile([C, E], F32, tag="weff", name="weff")
            nc.vector.tensor_add(weff[:], gw[:, 0, :], gw[:, 1, :])
            # fold the 1/HW of the y-mean into the gate weight
            nc.vector.tensor_scalar_mul(weff[:], weff[:], 1.0 / float(H * W))

            # ---- expert transpose with in-PE fold --------------------------
            # agg lhsT layout [c, t, o]; fold of the duplicated input halves
            # done by two accumulating PE transposes per tap (no DVE fold).
            wfts = []
            for e in range(E):
                we = wes[e]
                wt = wft.tile([C, K * K, O], F32, tag="wft", name=f"wft{e}")
                for t, (ky, kx) in enumerate(TAPS):
                    pst = psp.tile([128, 128], F32, tag="ps", name=f"pst{e}_{t}")
                    nc.tensor.matmul(pst[:], we[:, 0:C, ky, kx], ident[:],
                                     is_transpose=True, start=True, stop=False)
                    nc.tensor.matmul(pst[:], we[:, C:2 * C, ky, kx], ident[:],
                                     is_transpose=True, start=False, stop=True)
                    nc.scalar.copy(wt[:, t, :], pst[:])
                wfts.append(wt)

            # ---- y reduction -----------------------------------------------
            yflat = y_d[:].rearrange("b c h w -> b c (h w)")
            ysums = []
            yparts = []

            def emit_y_loads(b, sp_chunks, gp_chunks):
                """Issue the y-chunk DMAs for sample b, striped across rings."""
                ycs = [None] * NYCH
                order = []
                a, g = 0, 0
                while a < len(sp_chunks) or g < len(gp_chunks):
                    if a < len(sp_chunks):
                        order.append((sp_chunks[a], nc.sync)); a += 1
                    if g < len(gp_chunks):
                        order.append((gp_chunks[g], nc.gpsimd)); g += 1
                for j, eng in order:
                    yc = ypool.tile([C, YCHUNK], F32, tag="yc", name=f"yc{b}_{j}")
                    chained_dma(eng, yc[:], yflat[b, :, j * YCHUNK:(j + 1) * YCHUNK])
                    ycs[j] = yc
                return ycs

            def emit_y_reduces(b, ycs, js):
                if b >= len(yparts):
                    ypart = gp.tile([C, NYCH], F32, tag="ypart", name=f"ypart{b}")
                    yparts.append(ypart)
                for j in js:
                    nc.vector.reduce_sum(yparts[b][:, j:j + 1], ycs[j][:],
                                         axis=mybir.AxisListType.X)

            def finish_ysum(b):
                ysum = gp.tile([C, 1], F32, tag="ysum", name=f"ysum{b}")
                nc.vector.reduce_sum(ysum[:], yparts[b][:], axis=mybir.AxisListType.X)
                ysums.append(ysum)

            # ---- q chunk staging -------------------------------------------
            # chunk tile: flat [2 zeros][row -1 .. row 16][2 zeros], rows
            # packed at stride W. Column wrap-around is fixed up by the
            # edge-correction matmuls in conv_chunk.
            xcs = {}

            def load_xc(b, ch, eng):
                xr_lo = max(0, CH_ROWS * ch - 1)
                xr_hi = min(H - 1, CH_ROWS * ch + CH_ROWS)
                nrows = xr_hi - xr_lo + 1
                j0 = xr_lo - (CH_ROWS * ch - 1)
                xc = xcp.tile([C, XCF], F32R, tag="xc", name=f"xc{b}_{ch}")
                nc.gpsimd.memset(xc[:, 0:2].bitcast(F32), 0.0)
                nc.gpsimd.memset(
                    xc[:, 2 + (CH_ROWS + 2) * W: 2 + (CH_ROWS + 2) * W + 2].bitcast(F32), 0.0)
                if ch == 0:
                    nc.gpsimd.memset(xc[:, 2:2 + W].bitcast(F32), 0.0)
                if ch == NCH - 1:
                    nc.gpsimd.memset(
                        xc[:, 2 + (CH_ROWS + 1) * W: 2 + (CH_ROWS + 2) * W].bitcast(F32), 0.0)
                chained_dma(
                    eng,
                    xc[:, 2 + j0 * W: 2 + (j0 + nrows) * W],
                    q_d[b, :, xr_lo:xr_hi + 1, :].rearrange("c h w -> c (h w)").bitcast(F32R),
                )
                xcs[(b, ch)] = xc

            # ---- gating + weight aggregation per sample --------------------
            aggs = []

            def gate_and_agg(b):
                finish_ysum(b)
                ps13 = pse.tile([1, E], F32, tag="pse", name=f"ps13_{b}")
                nc.tensor.matmul(ps13[:], ysums[b][:], weff[:], start=True, stop=True)
                logits = gp.tile([1, E], F32, tag="logits", name=f"logits{b}")
                nc.vector.tensor_add(logits[:], ps13[:], gbt[:])
                mx = gp.tile([1, 1], F32, tag="mx", name=f"mx{b}")
                nc.vector.reduce_max(mx[:], logits[:], axis=mybir.AxisListType.X)
                nc.vector.tensor_scalar_mul(mx[:], mx[:], -1.0)
                nc.scalar.activation(logits[:], logits[:], mybir.ActivationFunctionType.Exp,
                                     bias=mx[:], scale=1.0)
                sm = gp.tile([1, 1], F32, tag="sm", name=f"sm{b}")
                nc.vector.reduce_sum(sm[:], logits[:], axis=mybir.AxisListType.X)
                nc.vector.reciprocal(sm[:], sm[:])
                nc.vector.tensor_scalar_mul(logits[:], logits[:], sm[:])
                # broadcast gates to all partitions via a K=1 matmul with ones
                psg = pse.tile([128, E], F32, tag="pse", name=f"psg{b}")
                nc.tensor.matmul(psg[:], ones[:], logits[:], start=True, stop=True)
                gbc = gp.tile([128, E], F32, tag="gbc", name=f"gbc{b}")
                nc.vector.tensor_copy(gbc[:], psg[:])

                # aggregate in 3 tap-groups so the first conv matmuls can
                # start while later groups still combine; per group a mul
                # plus two DVE FMAs (scalar_tensor_tensor)
                accf = atmp.tile([C, K * K, O], F32, tag="accf", name=f"accf{b}")
                agg = aggp.tile([C, K * K, O], F32R, tag="agg", name=f"agg{b}")
                for g3 in range(3):
                    sl = slice(3 * g3, 3 * g3 + 3)
                    nc.vector.tensor_scalar_mul(accf[:, sl, :], wfts[0][:, sl, :],
                                                gbc[:, 0:1])
                    nc.vector.scalar_tensor_tensor(
                        accf[:, sl, :], wfts[1][:, sl, :], gbc[:, 1:2],
                        accf[:, sl, :], MUL, ADD)
                    nc.vector.scalar_tensor_tensor(
                        agg[:, sl, :], wfts[2][:, sl, :], gbc[:, 2:3],
                        accf[:, sl, :], MUL, ADD)
                aggs.append(agg)

            # ---- conv ------------------------------------------------------
            # Main taps read the flat chunk at offset 2 + (4rb+ky)*W + kx-1.
            # For kx=0 the first column of each row wrongly reads the last
            # element of the previous row (and vice versa for kx=2), which
            # SAME-padding says should be zero.  err matmuls compute exactly
            # those wrong contributions; they are subtracted on the SBUF copy.
            def conv_chunk(b, ch):
                last = (b == BPC - 1) and (ch == NCH - 1)
                xc = xcs[(b, ch)]
                x1 = xc[:, 1:1 + (CH_ROWS + 2) * W].rearrange("c (r w) -> c r w", w=W)
                x2 = xc[:, 2:2 + (CH_ROWS + 3) * W].rearrange("c (r w) -> c r w", w=W)
                # err psum [O, 2, CH_ROWS]: group 0 = col 0, group 1 = col W-1
                errps = pse.tile([O, 2, CH_ROWS], F32, tag="pse", name=f"eps{b}_{ch}")
                first = True
                for t, (ky, kx) in enumerate(TAPS):
                    if kx == 1:
                        continue
                    if kx == 0:
                        g, rhs = 0, x1[:, ky:ky + CH_ROWS, 0:1]
                    else:
                        g, rhs = 1, x2[:, ky + 1:ky + 1 + CH_ROWS, 0:1]
                    nc.tensor.matmul(
                        errps[:, g, :], aggs[b][:, t, :], rhs,
                        start=first, stop=(t == len(TAPS) - 1), skip_group_check=True,
                    )
                    first = False
                osb = osbp.tile([O, CH_ROWS, W], F32, tag="osb", name=f"osb{b}_{ch}")
                for rb in range(NRB):
                    ps = psp.tile([O, RB_ROWS, W], F32, tag="ps", name=f"ps{b}_{ch}_{rb}")
                    for t, (ky, kx) in enumerate(TAPS):
                        jb = RB_ROWS * rb + ky
                        off = 2 + jb * W + kx - 1
                        rhs = xc[:, off:off + RB_ROWS * W]  # contiguous 512
                        nc.tensor.matmul(
                            ps[:],
                            aggs[b][:, t, :],
                            rhs,
                            start=(t == 0),
                            stop=(t == len(TAPS) - 1),
                        )
                    sl = slice(RB_ROWS * rb, RB_ROWS * (rb + 1))
                    nc.scalar.copy(osb[:, sl, :], ps[:])
                    if last:
                        # drain the tail chunk per row-block to cut the drain
                        nc.vector.tensor_sub(osb[:, sl, 0], osb[:, sl, 0],
                                             errps[:, 0, sl])
                        nc.vector.tensor_sub(osb[:, sl, W - 1], osb[:, sl, W - 1],
                                             errps[:, 1, sl])
                        r0 = CH_ROWS * ch + RB_ROWS * rb
                        nc.scalar.dma_start(out_d[b, :, r0:r0 + RB_ROWS, :],
                                            osb[:, sl, :])
                if not last:
                    nc.vector.tensor_sub(osb[:, :, 0], osb[:, :, 0], errps[:, 0, :])
                    nc.vector.tensor_sub(osb[:, :, W - 1], osb[:, :, W - 1],
                                         errps[:, 1, :])
                    r0 = CH_ROWS * ch
                    nc.scalar.dma_start(out_d[b, :, r0:r0 + CH_ROWS, :], osb[:])

            # ---- schedule --------------------------------------------------
            # Startup: both load rings carry experts then y0 (balanced to
            # ~6.4 MB each) then the first q chunks; the sample-0 gating
            # chain runs on DVE, unobstructed.
            y0_cs = emit_y_loads(0, sp_chunks=list(range(8)),
                                 gp_chunks=list(range(8, 16)))
            load_xc(0, 0, nc.sync)
            load_xc(0, 1, nc.gpsimd)
            load_xc(0, 2, nc.sync)
            load_xc(0, 3, nc.gpsimd)
            emit_y_reduces(0, y0_cs, range(NYCH))
            gate_and_agg(0)
            conv_chunk(0, 0)
            load_xc(0, 4, nc.sync)
            load_xc(0, 5, nc.gpsimd)
            conv_chunk(0, 1)
            # y1 rides both rings behind sample-0's early chunks
            y1_cs = emit_y_loads(1, sp_chunks=list(range(8)),
                                 gp_chunks=list(range(8, 16)))
            load_xc(0, 6, nc.sync)
            load_xc(0, 7, nc.gpsimd)
            conv_chunk(0, 2)
            emit_y_reduces(1, y1_cs, range(0, 6))
            conv_chunk(0, 3)
            emit_y_reduces(1, y1_cs, range(6, 12))
            load_xc(1, 0, nc.sync)
            load_xc(1, 1, nc.gpsimd)
            conv_chunk(0, 4)
            emit_y_reduces(1, y1_cs, range(12, 16))
            gate_and_agg(1)
            load_xc(1, 2, nc.sync)
            load_xc(1, 3, nc.gpsimd)
            conv_chunk(0, 5)
            load_xc(1, 4, nc.sync)
            load_xc(1, 5, nc.gpsimd)
            conv_chunk(0, 6)
            load_xc(1, 6, nc.sync)
            load_xc(1, 7, nc.gpsimd)
            conv_chunk(0, 7)
            for ch in range(NCH):
                conv_chunk(1, ch)

    nc.compile()
    return nc


_NC_CACHE = None


def kernel(q, y, experts, gate_w, gate_b, _trace=False, _result_box=None):
    global _NC_CACHE
    if _NC_CACHE is None:
        _NC_CACHE = build_nc()
    nc = _NC_CACHE

    q = np.ascontiguousarray(q, dtype=np.float32)
    y = np.ascontiguousarray(y, dtype=np.float32)
    experts = np.ascontiguousarray(experts, dtype=np.float32)
    gate_w = np.ascontiguousarray(gate_w, dtype=np.float32)
    gate_b = np.ascontiguousarray(gate_b, dtype=np.float32)

    in_maps = []
    for i in range(NCORES):
        sl = slice(i * BPC, (i + 1) * BPC)
        in_maps.append({
            "q": q[sl], "y": y[sl],
            "experts": experts, "gate_w": gate_w, "gate_b": gate_b,
        })

    kwargs = {}
    if _trace:
        kwargs = dict(trace=True, trace_cores=[0])
    res = run_bass_kernel_spmd(nc, in_maps, core_ids=list(range(NCORES)), **kwargs)
    if _result_box is not None:
        _result_box.append(res)
    return np.concatenate([res.results[i]["out"] for i in range(NCORES)], axis=0)
